# revision 41
# baseline (speedup 1.0000x reference)
"""Trainium2 Bass/Tile SPMD kernel for a 3-layer multimodal LightGCN-style
GNN (segment-sum SpMM message passing + BPR batch lookups).

Strategy (8 NeuronCores):
  - Rows (nodes) are sharded uniformly: core c owns rows [c*12500, (c+1)*12500).
  - Edges are sorted by destination row on the host and assigned to the
    owning core; each core computes its own output rows exactly (no
    cross-core reduction of partial sums).
  - The three feature matrices (E0, image_weight.T, text_weight.T) are
    concatenated into one [N, 192] table, stored bf16 padded to 256 cols
    (dma_gather needs 256B-multiple rows) so each layer is one SpMM with
    half the f32 gather traffic.
  - Edges are processed in 128-edge tiles (one-hot(localrow)*val built on
    DVE in bf16 -> bf16 PE matmul accumulating into the block's f32 PSUM).
    Gathers are segment-major: each dma_gather call covers up to TPG=8
    tiles spanning consecutive row blocks, amortizing the ~1us/call SWDGE
    descriptor-prep that dominates gather cost on HW.
  - After each of layers 1,2 an AllGather replicates the new bf16 table to
    all cores for the next layer's gathers.
  - Per-layer bias folds into the host-precomputed S initializer (S_final
    = X0 + 3*bias + sum ps_l), so the device only does S += ps per block,
    plus one ps+bias -> bf16 cast per block for the next layer's table.
  - S is held modality-major ([P, 3*NB*64]) so the final phase (mean +
    l2-normalized modality fusion) runs as ~20 whole-tensor DVE ops
    (grouped reductions + stride-0 broadcast multiplies) instead of
    ~1300 tiny per-block ops, then one 4D-AP DMA writes F, a final
    AllGather replicates it, and data-parallel batch index gathers
    produce the output.

Host-side work is layout prep only: dtype casts, edge sort/partition and
per-core input slab assembly. All accumulation happens on device in f32.
"""
import os
import sys

import numpy as np
import ml_dtypes

for _p in ("/opt/trn_rl_repo", "/root/.axon_site/_ro/trn_rl_repo"):
    if os.path.isdir(_p) and _p not in sys.path:
        sys.path.append(_p)

import concourse.bass as bass
import concourse.bacc as bacc
import concourse.mybir as mybir
import concourse.tile as tile
from concourse.bass_utils import run_bass_kernel_spmd

P = 128
BF16 = ml_dtypes.bfloat16


class Cfg:
    def __init__(self, n_users=50000, n_items=50000, embed=64, n_layers=3,
                 batch=4096, n_cores=8, cat_rate=0.02, seg_rows=25000,
                 tiles_per_gather=8, gather_kind="swdge"):
        self.gather_kind = gather_kind          # "swdge" | "indirect"
        self.elem_probe = None                  # override gather elem_size
        self.n_users = n_users
        self.n_items = n_items
        self.N = n_users + n_items
        self.embed = embed
        self.D = 3 * embed                      # 192: payload feature dim
        self.DP = 256                           # padded table row (bf16)
        self.n_layers = n_layers
        self.batch = batch
        self.NC = n_cores
        self.cat_rate = cat_rate
        assert self.N % n_cores == 0
        self.RPC = self.N // n_cores            # rows per core
        self.NB = (self.RPC + P - 1) // P       # row blocks per core
        self.NBP = self.NB * P                  # padded rows per core
        self.BPC = batch // n_cores             # batch elems per core
        assert self.BPC % P == 0
        self.NT = n_cores * self.NBP            # padded table rows (position space)
        self.SEG = 2 * self.NBP                 # table rows per gather segment
        assert self.SEG <= 32768
        self.NSEG = self.NT // self.SEG         # 4: one per 2 cores
        self.TPG = tiles_per_gather             # 128-edge tiles per dma_gather


def _balance_rows(cfg, rows, cols):
    """Assign each node a (core, block, lane) position so that per
    (core, col-segment, block) edge counts stay <= 512 (4 tiles) where
    possible. Nodes keep their natural 25000-row segment (rows of natural
    segment sig go to cores 2*sig / 2*sig+1), so a column's gather segment
    remains its natural one. Returns pos[r] = core*NBP + block*128 + lane."""
    N, NC, NB = cfg.N, cfg.NC, cfg.NB
    NSEG = cfg.NSEG
    NATSEG = N // NSEG                          # 25000 natural rows/segment
    CAP = 4 * P                                 # target edges per bucket
    # per-row in-degree split by the *column's* natural segment
    dkey = rows * NSEG + (cols // NATSEG)
    indeg = np.bincount(dkey, minlength=N * NSEG).reshape(N, NSEG)
    pcore = np.zeros(N, np.int64)
    pblk = np.zeros(N, np.int64)
    plane = np.zeros(N, np.int64)
    NBINS = 2 * NB                              # two cores per natural seg
    for sig in range(NSEG):
        r0 = sig * NATSEG
        d = indeg[r0:r0 + NATSEG].astype(np.float64)    # [NATSEG, 4]
        order = np.argsort(-d.sum(1), kind="stable")
        binsum = np.zeros((NBINS, NSEG))
        bincnt = np.zeros(NBINS, np.int64)
        binrows = [[] for _ in range(NBINS)]
        inf = np.float64(np.inf)
        assign = np.zeros(NATSEG, np.int64)
        for i in order:
            di = d[i]
            proj = binsum + di
            over = np.maximum(proj - CAP, 0.0).sum(1)
            cost = over * 1e6 + (proj * proj).sum(1) * 0.01 + bincnt * 1e-3
            cost[bincnt >= P] = inf
            j = int(np.argmin(cost))
            binsum[j] += di
            assign[i] = j
            bincnt[j] += 1

        # swap refinement: trade rows between over-cap and roomy bins
        # (counts preserved, so lane capacity stays satisfied)
        def ovf(x):
            return np.maximum(x - CAP, 0).sum(-1)
        for _ in range(8):
            over_bins = np.where((binsum > CAP).any(1))[0]
            if len(over_bins) == 0:
                break
            moved = 0
            for j in over_bins:
                if not (binsum[j] > CAP).any():
                    continue
                s = int(np.argmax(binsum[j]))
                rows_j = np.where(assign == j)[0]
                cand_i = rows_j[np.argsort(-d[rows_j, s])][:8]
                for i in cand_i:
                    di = d[i]
                    nj = binsum[j] - di + d
                    nm = binsum[assign] + di - d
                    gain = (ovf(binsum[j]) + ovf(binsum[assign])
                            - ovf(nj) - ovf(nm))
                    gain[assign == j] = -1
                    k = int(np.argmax(gain))
                    if gain[k] > 1e-9:
                        m = assign[k]
                        binsum[j] += d[k] - di
                        binsum[m] += di - d[k]
                        assign[i] = m
                        assign[k] = j
                        moved += 1
                        if (binsum[j] <= CAP).all():
                            break
            if moved == 0:
                break

        for j in range(NBINS):
            rs = r0 + np.where(assign == j)[0]
            pcore[rs] = 2 * sig + j // NB
            pblk[rs] = j % NB
            plane[rs] = np.arange(len(rs))
    return pcore, pblk, plane


def preprocess(cfg, inputs):
    """Host layout prep. Returns (meta, in_maps)."""
    N, D, E64 = cfg.N, cfg.D, cfg.embed
    NC, NB, SEG, NSEG = cfg.NC, cfg.NB, cfg.SEG, cfg.NSEG
    NBP, NT = cfg.NBP, cfg.NT
    NATSEG = N // NSEG

    rows = np.asarray(inputs["adj_rows"]).astype(np.int64)
    cols = np.asarray(inputs["adj_cols"]).astype(np.int64)
    vals = np.asarray(inputs["adj_vals"]).astype(np.float32)
    E0 = np.asarray(inputs["E0"]).astype(np.float32)
    iw = np.asarray(inputs["image_weight"]).astype(np.float32)
    ib = np.asarray(inputs["image_bias"]).astype(np.float32)
    tw = np.asarray(inputs["text_weight"]).astype(np.float32)
    tb = np.asarray(inputs["text_bias"]).astype(np.float32)
    uidx = np.asarray(inputs["user_indices"]).astype(np.int64)
    pidx = np.asarray(inputs["pos_item_indices"]).astype(np.int64)
    nidx = np.asarray(inputs["neg_item_indices"]).astype(np.int64)

    X0 = np.concatenate([E0, iw.T, tw.T], axis=1).astype(np.float32)
    bias192 = np.concatenate([np.zeros(E64, np.float32), ib, tb])
    bias_full = np.broadcast_to(bias192[None, :], (P, D)).copy()
    iota_bf = np.broadcast_to(
        np.arange(P, dtype=np.float32)[None, :], (P, P)).astype(BF16)

    # balanced node placement: pos[r] in [0, NT)
    pcore, pblk, plane = _balance_rows(cfg, rows, cols)
    pos = pcore * NBP + pblk * P + plane

    xt_bf = np.zeros((NT, cfg.DP), BF16)
    xt_bf[pos, :D] = X0.astype(BF16)

    # sort edges by (dest core, col segment, dest block): segment-major slabs
    seg_of = cols // NATSEG
    core_of = pcore[rows]
    blk_of = pblk[rows]
    order = np.lexsort((blk_of, seg_of, core_of))
    rows_s = rows[order]
    cols_s = cols[order]
    vals_s = vals[order]

    key_s = (core_of * NSEG + seg_of) * NB + blk_of
    key_s = key_s[order]

    # edge count for (core, segment, block)
    counts = np.zeros((NC, NSEG, NB), np.int64)
    starts = np.zeros((NC, NSEG, NB), np.int64)
    bnd = np.searchsorted(key_s, np.arange(NC * NSEG * NB + 1))
    starts.reshape(-1)[:] = bnd[:-1]
    counts.reshape(-1)[:] = np.diff(bnd)

    # shared (across cores) num_idxs per (segment, block), multiple of 128
    ni = -(-counts.max(axis=0) // P) * P            # [NSEG, NB]
    # ensure at least one tile per block so PSUM gets initialized
    for b in range(NB):
        if ni[:, b].sum() == 0:
            ni[0, b] = P
    T = int(ni.sum()) // P                          # total tiles per core

    meta = dict(ni=ni, T=T)

    XCOLS = int(ni.sum()) // 16
    in_maps = []
    for c in range(NC):
        # pad indices point at row 0 (valid data; one-hot val=0 kills the
        # contribution; skip-pads would leave stale SBUF lanes that can
        # hold NaN bit patterns, and 0*NaN = NaN poisons the PE matmul).
        idx16 = np.zeros((int(ni.sum()),), np.int16)
        vals_sl = np.zeros((T * P,), np.float32)
        lrow_sl = np.zeros((T * P,), np.float32)
        io = 0   # index offset (in idx units)
        to = 0   # tile offset
        for s in range(NSEG):
            for b in range(NB):
                nis = int(ni[s, b])
                if nis == 0:
                    continue
                st, cnt = starts[c, s, b], counts[c, s, b]
                idx16[io:io + cnt] = \
                    (pos[cols_s[st:st + cnt]] - s * SEG).astype(np.int16)
                vals_sl[to * P + np.arange(cnt)] = vals_s[st:st + cnt]
                lrow_sl[to * P + np.arange(cnt)] = plane[rows_s[st:st + cnt]]
                io += nis
                to += nis // P
        # wrap idx16 into [16, XCOLS] then replicate to 128 partitions
        idxw = idx16.reshape(XCOLS, 16).T
        idx_full = np.tile(idxw, (8, 1))
        # slab layout [P, T] (f32: tensor_scalar scalar APs must be f32)
        vals_sl = vals_sl.reshape(T, P).T.copy()
        lrow_sl = lrow_sl.reshape(T, P).T.copy()

        # S initializer, modality-major [P, 3*NB*64]:
        # X0 rows + 3*bias (bias folds out of the recursion)
        x0c = np.ones((NBP, D), np.float32)
        cmask = pcore == c
        x0c[(pblk[cmask] * P + plane[cmask])] = X0[cmask]
        x0c += cfg.n_layers * bias192[None, :]
        x0loc = np.ascontiguousarray(
            x0c.reshape(NB, P, 3, E64).transpose(1, 2, 0, 3)
            .reshape(P, 3 * NB * E64))

        # batch indices -> rows of the padded Ffull [NT, D]
        ntb = cfg.BPC // P
        fidx = np.zeros((P, 3 * ntb), np.int32)
        for s_i, arr in enumerate((uidx, cfg.n_users + pidx, cfg.n_users + nidx)):
            sl = arr[c * cfg.BPC:(c + 1) * cfg.BPC]
            gl = pos[sl]
            fidx[:, s_i * ntb:(s_i + 1) * ntb] = gl.reshape(ntb, P).T
        in_maps.append({
            "gidx": np.ascontiguousarray(idx_full),
            "evals": vals_sl,
            "lrow": lrow_sl,
            "biasf": bias_full,
            "iota": iota_bf,
            "x0loc": x0loc,
            "fidx": fidx.astype(np.int32),
            "xt": xt_bf,
        })
    return meta, in_maps


def build_program(cfg, meta, mode="full"):
    """Build the SPMD Bass program shared by all cores.
    mode gates pieces for timing probes: "full" | "nogather" | "nocompute"
    | "collonly" | "colls3" | "finalonly" | "empty"."""
    N, D, DP = cfg.N, cfg.D, cfg.DP
    NC, RPC, NB, SEG, NSEG = cfg.NC, cfg.RPC, cfg.NB, cfg.SEG, cfg.NSEG
    NBP, NT = cfg.NBP, cfg.NT
    NL = cfg.n_layers
    E64 = cfg.embed
    M = NB * E64                                 # cols per modality in S
    ni = meta["ni"]
    T = meta["T"]
    XCOLS = int(ni.sum()) // 16
    ntb = cfg.BPC // P
    f32 = mybir.dt.float32
    bf16 = mybir.dt.bfloat16

    # ---- gather call plan (shared across cores; from shared ni) ----
    # global tile index: s-major, block-sub order (matches host slabs)
    tile_base = np.zeros((NSEG, NB), np.int64)
    idx_base = np.zeros((NSEG, NB), np.int64)
    acc = 0
    for s in range(NSEG):
        for b in range(NB):
            tile_base[s, b] = acc
            idx_base[s, b] = acc * P
            acc += int(ni[s, b]) // P
    calls = []   # (first_b, s, idx_off, tiles=[(b, tidx), ...])
    for s in range(NSEG):
        stream = []
        for b in range(NB):
            for k in range(int(ni[s, b]) // P):
                stream.append((b, int(tile_base[s, b]) + k))
        for i in range(0, len(stream), cfg.TPG):
            chunk = stream[i:i + cfg.TPG]
            off = int(idx_base[s, 0]) + i * P
            calls.append((chunk[0][0], s, off, chunk))
    calls.sort(key=lambda c: (c[0], c[1]))

    nc = bacc.Bacc("TRN2", num_devices=NC, debug=False)
    xt = nc.dram_tensor("xt", [NT, DP], bf16, kind="ExternalInput")
    gidx = nc.dram_tensor("gidx", [P, XCOLS], mybir.dt.int16,
                          kind="ExternalInput")
    evals = nc.dram_tensor("evals", [P, T], f32, kind="ExternalInput")
    lrow = nc.dram_tensor("lrow", [P, T], f32, kind="ExternalInput")
    biasf = nc.dram_tensor("biasf", [P, D], f32, kind="ExternalInput")
    iota = nc.dram_tensor("iota", [P, P], bf16, kind="ExternalInput")
    x0loc = nc.dram_tensor("x0loc", [P, 3 * M], f32, kind="ExternalInput")
    fidx = nc.dram_tensor("fidx", [P, 3 * ntb], mybir.dt.int32,
                          kind="ExternalInput")
    bout = nc.dram_tensor("bout", [cfg.BPC, 3 * D], f32, kind="ExternalOutput")

    rg = [list(range(NC))]

    with tile.TileContext(nc) as tc:
        with tc.tile_pool(name="const", bufs=1) as cpool, \
             tc.tile_pool(name="g", bufs=12) as gpool, \
             tc.tile_pool(name="h", bufs=10) as hpool, \
             tc.tile_pool(name="e", bufs=3) as epool, \
             tc.tile_pool(name="f", bufs=1) as fpool, \
             tc.tile_pool(name="bg", bufs=2) as bgpool, \
             tc.tile_pool(name="ps", bufs=8, space="PSUM") as pspool, \
             tc.tile_pool(name="dram", bufs=1, space="DRAM") as dram:
            gidx_sb = cpool.tile([P, XCOLS], mybir.dt.int16)
            nc.sync.dma_start(out=gidx_sb[:], in_=gidx[:])
            evals_sb = cpool.tile([P, T], f32)
            nc.sync.dma_start(out=evals_sb[:], in_=evals[:])
            lrow_sb = cpool.tile([P, T], f32)
            nc.sync.dma_start(out=lrow_sb[:], in_=lrow[:])
            bias_sb = cpool.tile([P, D], f32)
            nc.sync.dma_start(out=bias_sb[:], in_=biasf[:])
            iota_sb = cpool.tile([P, P], bf16)
            nc.sync.dma_start(out=iota_sb[:], in_=iota[:])
            S_sb = cpool.tile([P, 3 * M], f32)
            nc.sync.dma_start(out=S_sb[:], in_=x0loc[:])
            fidx_sb = cpool.tile([P, 3 * ntb], mybir.dt.int32)
            nc.sync.dma_start(out=fidx_sb[:], in_=fidx[:])
            if mode == "nogather":
                dum_sb = cpool.tile([P, D], bf16)
                nc.vector.memset(dum_sb[:], 0.25)

            ag_in = dram.tile([NBP, DP], bf16)
            Xa = dram.tile([NT, DP], bf16, addr_space="Shared")
            Xb = dram.tile([NT, DP], bf16, addr_space="Shared")
            Fin = dram.tile([NBP, D], f32)
            Ffull = dram.tile([NT, D], f32, addr_space="Shared")

            sources = [xt, Xa, Xb]

            run_blocks = mode in ("full", "nogather", "nocompute")
            run_layer_ag = mode in ("full", "nogather", "nocompute",
                                    "collonly", "colls3")
            run_final_phase = mode in ("full", "nogather", "nocompute",
                                       "collonly", "finalonly")
            run_final_ag = mode not in ("empty",)
            run_batch = mode not in ("empty", "colls3")

            for layer in range(NL):
                src = sources[layer]
                tiles_of_block = [[] for _ in range(NB)]
                if run_blocks and mode != "nogather":
                    for (_, s, off, chunk) in calls:
                        tls = len(chunk)
                        cni = tls * P
                        gt = gpool.tile([P, tls * DP], bf16, tag="g")
                        nc.gpsimd.dma_gather(
                            out_ap=gt[:].rearrange("p (t e) -> p t e", t=tls),
                            in_ap=src[s * SEG:(s + 1) * SEG, :],
                            idxs_ap=gidx_sb[:, off // 16:(off + cni) // 16],
                            num_idxs=cni,
                            num_idxs_reg=cni,
                            elem_size=DP,
                        )
                        for slot, (b, tidx) in enumerate(chunk):
                            tiles_of_block[b].append((gt, slot, tidx))
                elif run_blocks:
                    for (_, s, off, chunk) in calls:
                        for slot, (b, tidx) in enumerate(chunk):
                            tiles_of_block[b].append((None, slot, tidx))
                for b in range(NB if run_blocks else 0):
                    gts = tiles_of_block[b]
                    nt = len(gts)
                    if mode == "nocompute":
                        continue
                    ps = pspool.tile([P, D], f32, space="PSUM", tag="ps")
                    # fused one-hot+scale: sh = (iota==lrow[t])*vals[t], bf16
                    for t in range(nt):
                        gt, kg, tidx = gts[t]
                        sh = hpool.tile([P, P], bf16, tag="sh")
                        nc.vector.tensor_scalar(
                            out=sh[:], in0=iota_sb[:],
                            scalar1=lrow_sb[:, tidx:tidx + 1],
                            scalar2=evals_sb[:, tidx:tidx + 1],
                            op0=mybir.AluOpType.is_equal,
                            op1=mybir.AluOpType.mult)
                        if mode == "nogather":
                            rhs = dum_sb[:]
                        else:
                            rhs = gt[:, kg * DP:kg * DP + D]
                        nc.tensor.matmul(
                            out=ps[:],
                            lhsT=sh[:],
                            rhs=rhs,
                            start=(t == 0), stop=(t == nt - 1))
                    # S[modality-major] += ps
                    Ssl = S_sb[:].rearrange("p (m nb k) -> p m nb k",
                                            m=3, nb=NB)[:, :, b, :]
                    nc.vector.tensor_tensor(
                        out=Ssl, in0=Ssl,
                        in1=ps[:].rearrange("p (m k) -> p m k", m=3),
                        op=mybir.AluOpType.add)
                    if layer < NL - 1:
                        xb = epool.tile([P, D], bf16, tag="xb")
                        nc.vector.tensor_tensor(out=xb[:], in0=ps[:],
                                                in1=bias_sb[:],
                                                op=mybir.AluOpType.add)
                        nc.sync.dma_start(
                            out=ag_in[b * P:(b + 1) * P, 0:D],
                            in_=xb[:])
                if layer < NL - 1 and run_layer_ag:
                    dst = sources[layer + 1]
                    nc.gpsimd.collective_compute(
                        "AllGather", mybir.AluOpType.bypass,
                        replica_groups=rg, ins=[ag_in[:]], outs=[dst[:]])

            # ---- final phase, vectorized over all blocks ----
            # S = [Se | Simg | Stxt], each [P, M]; M = NB*64, col b*64+k
            # F_comb = Se/4 + cat*(Simg/||img||) + cat*(Stxt/||txt||)
            #        = Se/4 + Simg*rinv_i + Stxt*rinv_t,
            #   rinv = 1/sqrt(ss/cat^2), ss = per-(row,block) sum of squares
            if run_final_phase:
                HM = M // 2
                rr = fpool.tile([P, 2 * NB], f32, tag="rr")
                sq = fpool.tile([P, HM], f32, tag="sq")
                for h in range(4):      # 4 half-modality chunks of img|txt
                    chunk = S_sb[:, M + h * HM:M + (h + 1) * HM]
                    nc.vector.tensor_tensor(out=sq[:], in0=chunk, in1=chunk,
                                            op=mybir.AluOpType.mult)
                    nc.vector.reduce_sum(
                        out=rr[:, h * (NB // 2):(h + 1) * (NB // 2)],
                        in_=sq[:].rearrange("p (nb k) -> p nb k", k=E64),
                        axis=mybir.AxisListType.X)
                sc = 1.0 / (cfg.cat_rate * cfg.cat_rate)
                nc.scalar.activation(out=rr[:], in_=rr[:],
                                     func=mybir.ActivationFunctionType.Sqrt,
                                     scale=sc)
                nc.vector.reciprocal(out=rr[:], in_=rr[:])
                # scale S by 1/(NL+1) in place (mean), then add the
                # normalized modal parts into the Se third
                inv = 1.0 / (NL + 1)
                cat4 = fpool.tile([P, HM], f32, tag="cat")
                for h in range(4):
                    chunk = S_sb[:, M + h * HM:M + (h + 1) * HM]
                    rsl = rr[:, h * (NB // 2):(h + 1) * (NB // 2)]
                    nc.vector.tensor_tensor(
                        out=cat4[:].rearrange("p (nb k) -> p nb k", k=E64),
                        in0=chunk.rearrange("p (nb k) -> p nb k", k=E64),
                        in1=rsl.rearrange("p (nb one) -> p nb one", one=1)
                            .broadcast_to((P, NB // 2, E64)),
                        op=mybir.AluOpType.mult)
                    esl = S_sb[:, (h % 2) * HM:(h % 2 + 1) * HM]
                    if h < 2:
                        nc.vector.tensor_scalar(out=esl, in0=esl,
                                                scalar1=inv, scalar2=None,
                                                op0=mybir.AluOpType.mult)
                    nc.vector.tensor_tensor(out=esl, in0=esl, in1=cat4[:],
                                            op=mybir.AluOpType.add)
                nc.vector.tensor_scalar(out=S_sb[:, M:3 * M],
                                        in0=S_sb[:, M:3 * M],
                                        scalar1=inv, scalar2=None,
                                        op0=mybir.AluOpType.mult)
                # 3 DMAs (one per modality): S[P, m, (nb k)] -> Fin rows
                for m in range(3):
                    nc.sync.dma_start(
                        out=Fin[:, m * E64:(m + 1) * E64]
                            .rearrange("(nb p) k -> p nb k", p=P),
                        in_=S_sb[:, m * M:(m + 1) * M]
                            .rearrange("p (nb k) -> p nb k", k=E64))
            if run_final_ag:
                nc.gpsimd.collective_compute(
                    "AllGather", mybir.AluOpType.bypass,
                    replica_groups=rg, ins=[Fin[:]], outs=[Ffull[:]])

            # ---- batch gathers ----
            for s_i in range(3 if run_batch else 0):
                for t in range(ntb):
                    gt = bgpool.tile([P, D], f32, tag="bg")
                    j = s_i * ntb + t
                    nc.gpsimd.indirect_dma_start(
                        out=gt[:], out_offset=None, in_=Ffull[:],
                        in_offset=bass.IndirectOffsetOnAxis(
                            ap=fidx_sb[:, j:j + 1], axis=0),
                    )
                    nc.sync.dma_start(
                        out=bout[t * P:(t + 1) * P, s_i * D:(s_i + 1) * D],
                        in_=gt[:])
    nc.compile()
    return nc


_CACHE = {}


def _get_program(cfg, meta):
    key = (meta["ni"].tobytes(), cfg.N, cfg.D, cfg.batch, cfg.NC)
    if key not in _CACHE:
        _CACHE[key] = build_program(cfg, meta)
    return _CACHE[key]


def run(cfg, inputs, runner=None):
    meta, in_maps = preprocess(cfg, inputs)
    nc = _get_program(cfg, meta)
    res = run_bass_kernel_spmd(nc, in_maps, core_ids=list(range(cfg.NC)))
    return assemble(cfg, res.results)


def assemble(cfg, results):
    D = cfg.D
    E64 = cfg.embed
    g = [np.concatenate([results[c]["bout"][:, s * D:(s + 1) * D]
                         for c in range(cfg.NC)], axis=0) for s in range(3)]
    out = []
    for part in range(3):          # combined, mean_img, mean_txt
        for s in range(3):         # user, pos, neg
            out.append(np.ascontiguousarray(g[s][:, part * E64:(part + 1) * E64]))
    return tuple(out)


def kernel(**inputs):
    cfg = Cfg()
    return run(cfg, inputs)


# revision 42
# speedup vs baseline: 1.7226x; 1.7226x over previous
"""Trainium2 Bass/Tile SPMD kernel for a 3-layer multimodal LightGCN-style
GNN (segment-sum SpMM message passing + BPR batch lookups).

Strategy (8 NeuronCores):
  - Rows (nodes) are sharded uniformly: core c owns rows [c*12500, (c+1)*12500).
  - Edges are sorted by destination row on the host and assigned to the
    owning core; each core computes its own output rows exactly (no
    cross-core reduction of partial sums).
  - The three feature matrices (E0, image_weight.T, text_weight.T) are
    concatenated into one [N, 192] table, stored bf16 padded to 256 cols
    (dma_gather needs 256B-multiple rows) so each layer is one SpMM with
    half the f32 gather traffic.
  - Edges are processed in 128-edge tiles (one-hot(localrow)*val built on
    DVE in bf16 -> bf16 PE matmul accumulating into the block's f32 PSUM).
    Gathers are segment-major: each dma_gather call covers up to TPG=8
    tiles spanning consecutive row blocks, amortizing the ~1us/call SWDGE
    descriptor-prep that dominates gather cost on HW.
  - After each of layers 1,2 an AllGather replicates the new bf16 table to
    all cores for the next layer's gathers.
  - Per-layer bias folds into the host-precomputed S initializer (S_final
    = X0 + 3*bias + sum ps_l), so the device only does S += ps per block,
    plus one ps+bias -> bf16 cast per block for the next layer's table.
  - S is held modality-major ([P, 3*NB*64]) so the final phase (mean +
    l2-normalized modality fusion) runs as ~20 whole-tensor DVE ops
    (grouped reductions + stride-0 broadcast multiplies) instead of
    ~1300 tiny per-block ops, then one 4D-AP DMA writes F, a final
    AllGather replicates it, and data-parallel batch index gathers
    produce the output.

Host-side work is layout prep only: dtype casts, edge sort/partition and
per-core input slab assembly. All accumulation happens on device in f32.
"""
import os
import sys

import numpy as np
import ml_dtypes

for _p in ("/opt/trn_rl_repo", "/root/.axon_site/_ro/trn_rl_repo"):
    if os.path.isdir(_p) and _p not in sys.path:
        sys.path.append(_p)

import concourse.bass as bass
import concourse.bacc as bacc
import concourse.mybir as mybir
import concourse.tile as tile
from concourse.bass_utils import run_bass_kernel_spmd

P = 128
BF16 = ml_dtypes.bfloat16


class Cfg:
    def __init__(self, n_users=50000, n_items=50000, embed=64, n_layers=3,
                 batch=4096, n_cores=8, cat_rate=0.02, seg_rows=25000,
                 tiles_per_gather=8, gather_kind="swdge"):
        self.gather_kind = gather_kind          # "swdge" | "indirect"
        self.elem_probe = None                  # override gather elem_size
        self.refine = True                      # swap-refinement in balance
        self.n_users = n_users
        self.n_items = n_items
        self.N = n_users + n_items
        self.embed = embed
        self.D = 3 * embed                      # 192: payload feature dim
        self.DP = 256                           # padded table row (bf16)
        self.n_layers = n_layers
        self.batch = batch
        self.NC = n_cores
        self.cat_rate = cat_rate
        assert self.N % n_cores == 0
        self.RPC = self.N // n_cores            # rows per core
        self.NB = (self.RPC + P - 1) // P       # row blocks per core
        self.NBP = self.NB * P                  # padded rows per core
        self.BPC = batch // n_cores             # batch elems per core
        assert self.BPC % P == 0
        self.NT = n_cores * self.NBP            # padded table rows (position space)
        self.SEG = 2 * self.NBP                 # table rows per gather segment
        assert self.SEG <= 32768
        self.NSEG = self.NT // self.SEG         # 4: one per 2 cores
        self.TPG = tiles_per_gather             # 128-edge tiles per dma_gather


def _balance_rows(cfg, rows, cols):
    """Assign each node a (core, block, lane) position so that per
    (core, col-segment, block) edge counts stay <= 512 (4 tiles) where
    possible. Nodes keep their natural 25000-row segment (rows of natural
    segment sig go to cores 2*sig / 2*sig+1), so a column's gather segment
    remains its natural one. Returns pos[r] = core*NBP + block*128 + lane."""
    N, NC, NB = cfg.N, cfg.NC, cfg.NB
    NSEG = cfg.NSEG
    NATSEG = N // NSEG                          # 25000 natural rows/segment
    CAP = 4 * P                                 # target edges per bucket
    # per-row in-degree split by the *column's* natural segment
    dkey = rows * NSEG + (cols // NATSEG)
    indeg = np.bincount(dkey, minlength=N * NSEG).reshape(N, NSEG)
    pcore = np.zeros(N, np.int64)
    pblk = np.zeros(N, np.int64)
    plane = np.zeros(N, np.int64)
    NBINS = 2 * NB                              # two cores per natural seg
    for sig in range(NSEG):
        r0 = sig * NATSEG
        d = indeg[r0:r0 + NATSEG].astype(np.float64)    # [NATSEG, 4]
        order = np.argsort(-d.sum(1), kind="stable")
        binsum = np.zeros((NBINS, NSEG))
        bincnt = np.zeros(NBINS, np.int64)
        binrows = [[] for _ in range(NBINS)]
        inf = np.float64(np.inf)
        assign = np.zeros(NATSEG, np.int64)
        for i in order:
            di = d[i]
            proj = binsum + di
            over = np.maximum(proj - CAP, 0.0).sum(1)
            cost = over * 1e6 + (proj * proj).sum(1) * 0.01 + bincnt * 1e-3
            cost[bincnt >= P] = inf
            j = int(np.argmin(cost))
            binsum[j] += di
            assign[i] = j
            bincnt[j] += 1

        # swap refinement: trade rows between over-cap and roomy bins
        # (counts preserved, so lane capacity stays satisfied)
        def ovf(x):
            return np.maximum(x - CAP, 0).sum(-1)
        for _ in range(8 if cfg.refine else 0):
            over_bins = np.where((binsum > CAP).any(1))[0]
            if len(over_bins) == 0:
                break
            moved = 0
            for j in over_bins:
                if not (binsum[j] > CAP).any():
                    continue
                s = int(np.argmax(binsum[j]))
                rows_j = np.where(assign == j)[0]
                cand_i = rows_j[np.argsort(-d[rows_j, s])][:8]
                for i in cand_i:
                    di = d[i]
                    nj = binsum[j] - di + d
                    nm = binsum[assign] + di - d
                    gain = (ovf(binsum[j]) + ovf(binsum[assign])
                            - ovf(nj) - ovf(nm))
                    gain[assign == j] = -1
                    k = int(np.argmax(gain))
                    if gain[k] > 1e-9:
                        m = assign[k]
                        binsum[j] += d[k] - di
                        binsum[m] += di - d[k]
                        assign[i] = m
                        assign[k] = j
                        moved += 1
                        if (binsum[j] <= CAP).all():
                            break
            if moved == 0:
                break

        for j in range(NBINS):
            rs = r0 + np.where(assign == j)[0]
            pcore[rs] = 2 * sig + j // NB
            pblk[rs] = j % NB
            plane[rs] = np.arange(len(rs))
    return pcore, pblk, plane


def preprocess(cfg, inputs):
    """Host layout prep. Returns (meta, in_maps)."""
    N, D, E64 = cfg.N, cfg.D, cfg.embed
    NC, NB, SEG, NSEG = cfg.NC, cfg.NB, cfg.SEG, cfg.NSEG
    NBP, NT = cfg.NBP, cfg.NT
    NATSEG = N // NSEG

    rows = np.asarray(inputs["adj_rows"]).astype(np.int64)
    cols = np.asarray(inputs["adj_cols"]).astype(np.int64)
    vals = np.asarray(inputs["adj_vals"]).astype(np.float32)
    E0 = np.asarray(inputs["E0"]).astype(np.float32)
    iw = np.asarray(inputs["image_weight"]).astype(np.float32)
    ib = np.asarray(inputs["image_bias"]).astype(np.float32)
    tw = np.asarray(inputs["text_weight"]).astype(np.float32)
    tb = np.asarray(inputs["text_bias"]).astype(np.float32)
    uidx = np.asarray(inputs["user_indices"]).astype(np.int64)
    pidx = np.asarray(inputs["pos_item_indices"]).astype(np.int64)
    nidx = np.asarray(inputs["neg_item_indices"]).astype(np.int64)

    X0 = np.concatenate([E0, iw.T, tw.T], axis=1).astype(np.float32)
    bias192 = np.concatenate([np.zeros(E64, np.float32), ib, tb])
    bias_full = np.broadcast_to(bias192[None, :], (P, D)).copy()
    iota_bf = np.broadcast_to(
        np.arange(P, dtype=np.float32)[None, :], (P, P)).astype(BF16)

    # balanced node placement: pos[r] in [0, NT)
    pcore, pblk, plane = _balance_rows(cfg, rows, cols)
    pos = pcore * NBP + pblk * P + plane

    xt_bf = np.zeros((NT, cfg.DP), BF16)
    xt_bf[pos, :D] = X0.astype(BF16)

    # sort edges by (dest core, col segment, dest block): segment-major slabs
    seg_of = cols // NATSEG
    core_of = pcore[rows]
    blk_of = pblk[rows]
    order = np.lexsort((blk_of, seg_of, core_of))
    rows_s = rows[order]
    cols_s = cols[order]
    vals_s = vals[order]

    key_s = (core_of * NSEG + seg_of) * NB + blk_of
    key_s = key_s[order]

    # edge count for (core, segment, block)
    counts = np.zeros((NC, NSEG, NB), np.int64)
    starts = np.zeros((NC, NSEG, NB), np.int64)
    bnd = np.searchsorted(key_s, np.arange(NC * NSEG * NB + 1))
    starts.reshape(-1)[:] = bnd[:-1]
    counts.reshape(-1)[:] = np.diff(bnd)

    # shared (across cores) num_idxs per (segment, block), multiple of 128
    ni = -(-counts.max(axis=0) // P) * P            # [NSEG, NB]
    # ensure at least one tile per block so PSUM gets initialized
    for b in range(NB):
        if ni[:, b].sum() == 0:
            ni[0, b] = P
    T = int(ni.sum()) // P                          # total tiles per core

    meta = dict(ni=ni, T=T)

    XCOLS = int(ni.sum()) // 16
    in_maps = []
    for c in range(NC):
        # pad indices point at row 0 (valid data; one-hot val=0 kills the
        # contribution; skip-pads would leave stale SBUF lanes that can
        # hold NaN bit patterns, and 0*NaN = NaN poisons the PE matmul).
        idx16 = np.zeros((int(ni.sum()),), np.int16)
        vals_sl = np.zeros((T * P,), np.float32)
        lrow_sl = np.zeros((T * P,), np.float32)
        io = 0   # index offset (in idx units)
        to = 0   # tile offset
        for s in range(NSEG):
            for b in range(NB):
                nis = int(ni[s, b])
                if nis == 0:
                    continue
                st, cnt = starts[c, s, b], counts[c, s, b]
                idx16[io:io + cnt] = \
                    (pos[cols_s[st:st + cnt]] - s * SEG).astype(np.int16)
                vals_sl[to * P + np.arange(cnt)] = vals_s[st:st + cnt]
                lrow_sl[to * P + np.arange(cnt)] = plane[rows_s[st:st + cnt]]
                io += nis
                to += nis // P
        # wrap idx16 into [16, XCOLS] then replicate to 128 partitions
        idxw = idx16.reshape(XCOLS, 16).T
        idx_full = np.tile(idxw, (8, 1))
        # slab layout [P, T] (f32: tensor_scalar scalar APs must be f32)
        vals_sl = vals_sl.reshape(T, P).T.copy()
        lrow_sl = lrow_sl.reshape(T, P).T.copy()

        # S initializer, modality-major [P, 3*NB*64]:
        # X0 rows + 3*bias (bias folds out of the recursion)
        x0c = np.ones((NBP, D), np.float32)
        cmask = pcore == c
        x0c[(pblk[cmask] * P + plane[cmask])] = X0[cmask]
        x0c += cfg.n_layers * bias192[None, :]
        x0loc = np.ascontiguousarray(
            x0c.reshape(NB, P, 3, E64).transpose(1, 2, 0, 3)
            .reshape(P, 3 * NB * E64))

        # batch indices -> rows of the padded Ffull [NT, D]
        ntb = cfg.BPC // P
        fidx = np.zeros((P, 3 * ntb), np.int32)
        for s_i, arr in enumerate((uidx, cfg.n_users + pidx, cfg.n_users + nidx)):
            sl = arr[c * cfg.BPC:(c + 1) * cfg.BPC]
            gl = pos[sl]
            fidx[:, s_i * ntb:(s_i + 1) * ntb] = gl.reshape(ntb, P).T
        in_maps.append({
            "gidx": np.ascontiguousarray(idx_full),
            "evals": vals_sl,
            "lrow": lrow_sl,
            "biasf": bias_full,
            "iota": iota_bf,
            "x0loc": x0loc,
            "fidx": fidx.astype(np.int32),
            "xt": xt_bf,
        })
    return meta, in_maps


def build_program(cfg, meta, mode="full"):
    """Build the SPMD Bass program shared by all cores.
    mode gates pieces for timing probes: "full" | "nogather" | "nocompute"
    | "collonly" | "colls3" | "finalonly" | "empty"."""
    N, D, DP = cfg.N, cfg.D, cfg.DP
    NC, RPC, NB, SEG, NSEG = cfg.NC, cfg.RPC, cfg.NB, cfg.SEG, cfg.NSEG
    NBP, NT = cfg.NBP, cfg.NT
    NL = cfg.n_layers
    E64 = cfg.embed
    M = NB * E64                                 # cols per modality in S
    ni = meta["ni"]
    T = meta["T"]
    XCOLS = int(ni.sum()) // 16
    ntb = cfg.BPC // P
    f32 = mybir.dt.float32
    bf16 = mybir.dt.bfloat16

    # ---- gather call plan (shared across cores; from shared ni) ----
    # global tile index: s-major, block-sub order (matches host slabs)
    tile_base = np.zeros((NSEG, NB), np.int64)
    idx_base = np.zeros((NSEG, NB), np.int64)
    acc = 0
    for s in range(NSEG):
        for b in range(NB):
            tile_base[s, b] = acc
            idx_base[s, b] = acc * P
            acc += int(ni[s, b]) // P
    calls = []   # (first_b, s, idx_off, tiles=[(b, tidx), ...])
    for s in range(NSEG):
        stream = []
        for b in range(NB):
            for k in range(int(ni[s, b]) // P):
                stream.append((b, int(tile_base[s, b]) + k))
        for i in range(0, len(stream), cfg.TPG):
            chunk = stream[i:i + cfg.TPG]
            off = int(idx_base[s, 0]) + i * P
            calls.append((chunk[0][0], s, off, chunk))
    calls.sort(key=lambda c: (c[0], c[1]))

    nc = bacc.Bacc("TRN2", num_devices=NC, debug=False)
    xt = nc.dram_tensor("xt", [NT, DP], bf16, kind="ExternalInput")
    gidx = nc.dram_tensor("gidx", [P, XCOLS], mybir.dt.int16,
                          kind="ExternalInput")
    evals = nc.dram_tensor("evals", [P, T], f32, kind="ExternalInput")
    lrow = nc.dram_tensor("lrow", [P, T], f32, kind="ExternalInput")
    biasf = nc.dram_tensor("biasf", [P, D], f32, kind="ExternalInput")
    iota = nc.dram_tensor("iota", [P, P], bf16, kind="ExternalInput")
    x0loc = nc.dram_tensor("x0loc", [P, 3 * M], f32, kind="ExternalInput")
    fidx = nc.dram_tensor("fidx", [P, 3 * ntb], mybir.dt.int32,
                          kind="ExternalInput")
    bout = nc.dram_tensor("bout", [cfg.BPC, 3 * D], f32, kind="ExternalOutput")

    rg = [list(range(NC))]

    with tile.TileContext(nc) as tc:
        with tc.tile_pool(name="const", bufs=1) as cpool, \
             tc.tile_pool(name="g", bufs=12) as gpool, \
             tc.tile_pool(name="h", bufs=10) as hpool, \
             tc.tile_pool(name="e", bufs=3) as epool, \
             tc.tile_pool(name="f", bufs=1) as fpool, \
             tc.tile_pool(name="bg", bufs=2) as bgpool, \
             tc.tile_pool(name="ps", bufs=8, space="PSUM") as pspool, \
             tc.tile_pool(name="dram", bufs=1, space="DRAM") as dram:
            gidx_sb = cpool.tile([P, XCOLS], mybir.dt.int16)
            nc.sync.dma_start(out=gidx_sb[:], in_=gidx[:])
            evals_sb = cpool.tile([P, T], f32)
            nc.sync.dma_start(out=evals_sb[:], in_=evals[:])
            lrow_sb = cpool.tile([P, T], f32)
            nc.sync.dma_start(out=lrow_sb[:], in_=lrow[:])
            bias_sb = cpool.tile([P, D], f32)
            nc.sync.dma_start(out=bias_sb[:], in_=biasf[:])
            iota_sb = cpool.tile([P, P], bf16)
            nc.sync.dma_start(out=iota_sb[:], in_=iota[:])
            S_sb = cpool.tile([P, 3 * M], f32)
            nc.sync.dma_start(out=S_sb[:], in_=x0loc[:])
            fidx_sb = cpool.tile([P, 3 * ntb], mybir.dt.int32)
            nc.sync.dma_start(out=fidx_sb[:], in_=fidx[:])
            if mode == "nogather":
                dum_sb = cpool.tile([P, D], bf16)
                nc.vector.memset(dum_sb[:], 0.25)

            ag_in = dram.tile([NBP, DP], bf16)
            Xa = dram.tile([NT, DP], bf16, addr_space="Shared")
            Xb = dram.tile([NT, DP], bf16, addr_space="Shared")
            Fin = dram.tile([NBP, D], f32)
            Ffull = dram.tile([NT, D], f32, addr_space="Shared")

            sources = [xt, Xa, Xb]

            run_blocks = mode in ("full", "nogather", "nocompute")
            run_layer_ag = mode in ("full", "nogather", "nocompute",
                                    "collonly", "colls3")
            run_final_phase = mode in ("full", "nogather", "nocompute",
                                       "collonly", "finalonly")
            run_final_ag = mode not in ("empty",)
            run_batch = mode not in ("empty", "colls3")

            for layer in range(NL):
                src = sources[layer]
                tiles_of_block = [[] for _ in range(NB)]
                if run_blocks and mode != "nogather":
                    for (_, s, off, chunk) in calls:
                        tls = len(chunk)
                        cni = tls * P
                        gt = gpool.tile([P, tls * DP], bf16, tag="g")
                        nc.gpsimd.dma_gather(
                            out_ap=gt[:].rearrange("p (t e) -> p t e", t=tls),
                            in_ap=src[s * SEG:(s + 1) * SEG, :],
                            idxs_ap=gidx_sb[:, off // 16:(off + cni) // 16],
                            num_idxs=cni,
                            num_idxs_reg=cni,
                            elem_size=DP,
                        )
                        for slot, (b, tidx) in enumerate(chunk):
                            tiles_of_block[b].append((gt, slot, tidx))
                elif run_blocks:
                    for (_, s, off, chunk) in calls:
                        for slot, (b, tidx) in enumerate(chunk):
                            tiles_of_block[b].append((None, slot, tidx))
                for b in range(NB if run_blocks else 0):
                    gts = tiles_of_block[b]
                    nt = len(gts)
                    if mode == "nocompute":
                        continue
                    ps = pspool.tile([P, D], f32, space="PSUM", tag="ps")
                    # fused one-hot+scale: sh = (iota==lrow[t])*vals[t], bf16
                    for t in range(nt):
                        gt, kg, tidx = gts[t]
                        sh = hpool.tile([P, P], bf16, tag="sh")
                        nc.vector.tensor_scalar(
                            out=sh[:], in0=iota_sb[:],
                            scalar1=lrow_sb[:, tidx:tidx + 1],
                            scalar2=evals_sb[:, tidx:tidx + 1],
                            op0=mybir.AluOpType.is_equal,
                            op1=mybir.AluOpType.mult)
                        if mode == "nogather":
                            rhs = dum_sb[:]
                        else:
                            rhs = gt[:, kg * DP:kg * DP + D]
                        nc.tensor.matmul(
                            out=ps[:],
                            lhsT=sh[:],
                            rhs=rhs,
                            start=(t == 0), stop=(t == nt - 1))
                    # S[modality-major] += ps
                    Ssl = S_sb[:].rearrange("p (m nb k) -> p m nb k",
                                            m=3, nb=NB)[:, :, b, :]
                    nc.vector.tensor_tensor(
                        out=Ssl, in0=Ssl,
                        in1=ps[:].rearrange("p (m k) -> p m k", m=3),
                        op=mybir.AluOpType.add)
                    if layer < NL - 1:
                        xb = epool.tile([P, D], bf16, tag="xb")
                        nc.vector.tensor_tensor(out=xb[:], in0=ps[:],
                                                in1=bias_sb[:],
                                                op=mybir.AluOpType.add)
                        nc.sync.dma_start(
                            out=ag_in[b * P:(b + 1) * P, 0:D],
                            in_=xb[:])
                if layer < NL - 1 and run_layer_ag:
                    dst = sources[layer + 1]
                    nc.gpsimd.collective_compute(
                        "AllGather", mybir.AluOpType.bypass,
                        replica_groups=rg, ins=[ag_in[:]], outs=[dst[:]])

            # ---- final phase, vectorized over all blocks ----
            # S = [Se | Simg | Stxt], each [P, M]; M = NB*64, col b*64+k
            # F_comb = Se/4 + cat*(Simg/||img||) + cat*(Stxt/||txt||)
            #        = Se/4 + Simg*rinv_i + Stxt*rinv_t,
            #   rinv = 1/sqrt(ss/cat^2), ss = per-(row,block) sum of squares
            if run_final_phase:
                HM = M // 2
                rr = fpool.tile([P, 2 * NB], f32, tag="rr")
                sq = fpool.tile([P, HM], f32, tag="sq")
                for h in range(4):      # 4 half-modality chunks of img|txt
                    chunk = S_sb[:, M + h * HM:M + (h + 1) * HM]
                    nc.vector.tensor_tensor(out=sq[:], in0=chunk, in1=chunk,
                                            op=mybir.AluOpType.mult)
                    nc.vector.reduce_sum(
                        out=rr[:, h * (NB // 2):(h + 1) * (NB // 2)],
                        in_=sq[:].rearrange("p (nb k) -> p nb k", k=E64),
                        axis=mybir.AxisListType.X)
                sc = 1.0 / (cfg.cat_rate * cfg.cat_rate)
                nc.scalar.activation(out=rr[:], in_=rr[:],
                                     func=mybir.ActivationFunctionType.Sqrt,
                                     scale=sc)
                nc.vector.reciprocal(out=rr[:], in_=rr[:])
                # scale S by 1/(NL+1) in place (mean), then add the
                # normalized modal parts into the Se third
                inv = 1.0 / (NL + 1)
                cat4 = fpool.tile([P, HM], f32, tag="cat")
                for h in range(4):
                    chunk = S_sb[:, M + h * HM:M + (h + 1) * HM]
                    rsl = rr[:, h * (NB // 2):(h + 1) * (NB // 2)]
                    nc.vector.tensor_tensor(
                        out=cat4[:].rearrange("p (nb k) -> p nb k", k=E64),
                        in0=chunk.rearrange("p (nb k) -> p nb k", k=E64),
                        in1=rsl.rearrange("p (nb one) -> p nb one", one=1)
                            .broadcast_to((P, NB // 2, E64)),
                        op=mybir.AluOpType.mult)
                    esl = S_sb[:, (h % 2) * HM:(h % 2 + 1) * HM]
                    if h < 2:
                        nc.vector.tensor_scalar(out=esl, in0=esl,
                                                scalar1=inv, scalar2=None,
                                                op0=mybir.AluOpType.mult)
                    nc.vector.tensor_tensor(out=esl, in0=esl, in1=cat4[:],
                                            op=mybir.AluOpType.add)
                nc.vector.tensor_scalar(out=S_sb[:, M:3 * M],
                                        in0=S_sb[:, M:3 * M],
                                        scalar1=inv, scalar2=None,
                                        op0=mybir.AluOpType.mult)
                # 3 DMAs (one per modality): S[P, m, (nb k)] -> Fin rows
                for m in range(3):
                    nc.sync.dma_start(
                        out=Fin[:, m * E64:(m + 1) * E64]
                            .rearrange("(nb p) k -> p nb k", p=P),
                        in_=S_sb[:, m * M:(m + 1) * M]
                            .rearrange("p (nb k) -> p nb k", k=E64))
            if run_final_ag:
                nc.gpsimd.collective_compute(
                    "AllGather", mybir.AluOpType.bypass,
                    replica_groups=rg, ins=[Fin[:]], outs=[Ffull[:]])

            # ---- batch gathers ----
            for s_i in range(3 if run_batch else 0):
                for t in range(ntb):
                    gt = bgpool.tile([P, D], f32, tag="bg")
                    j = s_i * ntb + t
                    nc.gpsimd.indirect_dma_start(
                        out=gt[:], out_offset=None, in_=Ffull[:],
                        in_offset=bass.IndirectOffsetOnAxis(
                            ap=fidx_sb[:, j:j + 1], axis=0),
                    )
                    nc.sync.dma_start(
                        out=bout[t * P:(t + 1) * P, s_i * D:(s_i + 1) * D],
                        in_=gt[:])
    nc.compile()
    return nc


_CACHE = {}


def _get_program(cfg, meta):
    key = (meta["ni"].tobytes(), cfg.N, cfg.D, cfg.batch, cfg.NC)
    if key not in _CACHE:
        _CACHE[key] = build_program(cfg, meta)
    return _CACHE[key]


def run(cfg, inputs, runner=None):
    meta, in_maps = preprocess(cfg, inputs)
    nc = _get_program(cfg, meta)
    res = run_bass_kernel_spmd(nc, in_maps, core_ids=list(range(cfg.NC)))
    return assemble(cfg, res.results)


def assemble(cfg, results):
    D = cfg.D
    E64 = cfg.embed
    g = [np.concatenate([results[c]["bout"][:, s * D:(s + 1) * D]
                         for c in range(cfg.NC)], axis=0) for s in range(3)]
    out = []
    for part in range(3):          # combined, mean_img, mean_txt
        for s in range(3):         # user, pos, neg
            out.append(np.ascontiguousarray(g[s][:, part * E64:(part + 1) * E64]))
    return tuple(out)


def kernel(**inputs):
    cfg = Cfg()
    return run(cfg, inputs)
